# revision 17
# baseline (speedup 1.0000x reference)
"""BoxE scorer kernel v3 for Trainium2 (8 NeuronCores, label-sharded).

Same widened-relu math as v2 (see below), but the per-group elementwise
work is load-balanced across THREE engines instead of two:

  DVE:   8x t'-op (x16*invhd - cod, fp16 4x-mode), grouped sign-bit AND,
         RT_DVE rt-ops                                   (~6.1 us/group)
  Pool:  RT_POOL rt-ops, grouped tensor-tensor square
         over the last (GRP-SQ_ACT) labels               (~5.5 us/group)
  Act:   grouped Square over the first SQ_ACT labels     (~5.4 us/group)
  PE:    2 matvecs per (label, chunk) + base matmuls     (cheap)

v2 ran the whole t'/AND/rt chain on DVE (7.9 us/group) with the full
grouped Square on Act (7.0 us/group); Pool sat idle.

Math ("widened-relu" formulation): with l1 = |x - cen|, hd = d/2,
m = relu(l1 - hd), s = [l1 > hd], the outside correction is
  corr_h = alpha*m^2 + beta*m + gamma*s .
Choosing w > 0 with  -alpha*w^2 + beta*w = gamma  and
rt = relu(l1 - hd + w):
  alpha*rt^2 + (beta - 2*alpha*w)*rt  ==  corr_h
exactly for points outside or deep inside; the band l1 in (hd-w, hd)
picks up a small spurious term bounded by gamma (measured end-to-end
fro err ~4e-3 against the fp64 reference; budget 2e-2).
Per label only
  rt' = relu(l1' - 1 + w')   (dual-op tensor_scalar, vec scalar bias)
  q   = rt'^2                (grouped Square)
plus TWO PE matvecs per (label, chunk) and the base quadratic
sum_h a^2 (x-cen)^2 as 3 dense f32 matmuls.
"""

from contextlib import ExitStack

import numpy as np

import concourse.bass as bass
import concourse.tile as tile
from concourse import bacc, mybir
from concourse import bass_utils

F32 = mybir.dt.float32
F16 = mybir.dt.float16
BF16 = mybir.dt.bfloat16
U16 = mybir.dt.uint16
A = mybir.AluOpType
ACT = mybir.ActivationFunctionType

B = 1024      # batch
H = 128       # hidden
L = 2048      # num labels
N_CORES = 8
LPC = L // N_CORES   # labels per core
NBCH = B // 128      # batch chunks of 128
GRP = 8              # labels per grouped instruction

RT_DVE = 8    # rt-op labels on DVE
SQ_ACT = 2    # square labels on Act grouped Square (rest on Pool tt-mult)
ABS_ACT = 3   # trailing labels whose |t'| comes from Act Abs (skip t'+AND)


def build_nc(repeat: int = 1, rt_dve: int = RT_DVE, sq_act: int = SQ_ACT,
             abs_act: int = ABS_ACT):
    nc = bacc.Bacc("TRN2", target_bir_lowering=False, debug=False,
                   num_devices=N_CORES)
    xT_d = nc.dram_tensor("xT", (H, B), F32, kind="ExternalInput")
    mnT_d = nc.dram_tensor("mnT", (H, LPC), F32, kind="ExternalInput")
    rawT_d = nc.dram_tensor("rawT", (H, LPC), F32, kind="ExternalInput")
    out_d = nc.dram_tensor("out", (B, LPC), F32, kind="ExternalOutput")

    with tile.TileContext(nc) as tc:
        with ExitStack() as ctx:
            cpool = ctx.enter_context(tc.tile_pool(name="consts", bufs=1))
            pspool = ctx.enter_context(
                tc.tile_pool(name="psum", bufs=1, space=bass.MemorySpace.PSUM))

            # ---- load inputs ----
            ppool_cm = tc.tile_pool(name="pre", bufs=1)
            ppool = ppool_cm.__enter__()
            rawT = ppool.tile([H, LPC], F32, tag="rawT")
            nc.sync.dma_start(rawT[:], rawT_d.ap())
            xT = cpool.tile([H, B], F32, tag="xT")
            nc.sync.dma_start(xT[:], xT_d.ap())
            mnT = ppool.tile([H, LPC], F32, tag="mnT")
            nc.sync.dma_start(mnT[:], mnT_d.ap())

            def f32t(tag, pool=None):
                return (pool or cpool).tile([H, LPC], F32, tag=tag, name=tag)

            # ---- per-label coefficients (all [H, LPC] f32) ----
            # Critical-path consts (t'-op needs invhd/cod; rt needs wm1)
            # first so the main loop can start while the rest finishes.
            # Independent chains alternate DVE/Pool.
            e = f32t("e", pool=ppool)
            nc.scalar.activation(e[:], rawT[:], ACT.Exp)
            e1 = f32t("e1", pool=ppool)
            nc.vector.tensor_scalar(e1[:], e[:], 1.0, None, A.add)
            delta = f32t("delta", pool=ppool)     # softplus(raw)
            nc.scalar.activation(delta[:], e1[:], ACT.Ln)

            hd = f32t("hd", pool=ppool)          # d/2
            nc.vector.tensor_scalar(hd[:], delta[:], 0.5, None, A.mult)
            cen = f32t("cen", pool=ppool)        # mn + d/2
            nc.gpsimd.tensor_tensor(cen[:], mnT[:], hd[:], A.add)
            invhd = f32t("invhd")                # 1/hd
            nc.vector.reciprocal(invhd[:], hd[:])
            cod = f32t("cod")                    # cen/hd
            nc.vector.tensor_tensor(cod[:], cen[:], invhd[:], A.mult)
            ncod = f32t("ncod")                  # -cen/hd (Act-Abs bias)
            nc.vector.tensor_scalar(ncod[:], cod[:], -1.0, None, A.mult)

            dp1 = f32t("dp1", pool=ppool)        # bb = d+1
            nc.vector.tensor_scalar(dp1[:], delta[:], 1.0, None, A.add)
            dp1e = f32t("dp1e", pool=ppool)
            nc.vector.tensor_scalar(dp1e[:], dp1[:], 1e-10, None, A.add)
            a_ = f32t("a_", pool=ppool)          # a = 1/(bb+1e-10)
            nc.vector.reciprocal(a_[:], dp1e[:])
            de = f32t("de", pool=ppool)
            nc.vector.tensor_scalar(de[:], delta[:], 1e-10, None, A.add)
            rd = f32t("rd", pool=ppool)          # 1/(d+1e-10)
            nc.vector.reciprocal(rd[:], de[:])

            dmr = f32t("dmr", pool=ppool)        # d - 1/d
            nc.gpsimd.tensor_tensor(dmr[:], delta[:], rd[:], A.subtract)
            nhd = f32t("nhd", pool=ppool)        # -d/2
            nc.vector.tensor_scalar(nhd[:], hd[:], -1.0, None, A.mult)
            c_ = f32t("c_", pool=ppool)          # c = -(d/2)(d - 1/d)
            nc.gpsimd.tensor_tensor(c_[:], dmr[:], nhd[:], A.mult)

            Dl = f32t("Dl", pool=ppool)          # D = bb - a
            nc.gpsimd.tensor_tensor(Dl[:], dp1[:], a_[:], A.subtract)
            abb = f32t("abb", pool=ppool)        # Q = bb + a
            nc.gpsimd.tensor_tensor(abb[:], dp1[:], a_[:], A.add)
            al = f32t("al", pool=ppool)          # alpha = D*Q
            nc.gpsimd.tensor_tensor(al[:], Dl[:], abb[:], A.mult)

            # w = hd + c/Q ;  w' = w/hd ; wm1 = w' - 1 (rt-op bias)
            rq = f32t("rq", pool=ppool)          # 1/Q
            nc.vector.reciprocal(rq[:], abb[:])
            cq_ = f32t("cq_", pool=ppool)        # c/Q
            nc.gpsimd.tensor_tensor(cq_[:], c_[:], rq[:], A.mult)
            w_ = f32t("w_", pool=ppool)          # w = hd + c/Q
            nc.gpsimd.tensor_tensor(w_[:], hd[:], cq_[:], A.add)
            wp = f32t("wp", pool=ppool)          # w' = w/hd
            nc.gpsimd.tensor_tensor(wp[:], w_[:], invhd[:], A.mult)
            wm1 = f32t("wm1")                    # w' - 1
            nc.vector.tensor_scalar(wm1[:], wp[:], 1.0, None, A.subtract)

            t2 = f32t("t2", pool=ppool)          # D + Q
            nc.gpsimd.tensor_tensor(t2[:], Dl[:], abb[:], A.add)
            t3 = f32t("t3", pool=ppool)          # c*(D+Q)
            nc.gpsimd.tensor_tensor(t3[:], t2[:], c_[:], A.mult)
            t4 = f32t("t4", pool=ppool)          # alpha*hd
            nc.gpsimd.tensor_tensor(t4[:], al[:], hd[:], A.mult)
            t5 = f32t("t5", pool=ppool)          # 2*alpha*hd
            nc.vector.tensor_scalar(t5[:], t4[:], 2.0, None, A.mult)
            bp = f32t("bp", pool=ppool)          # beta
            nc.gpsimd.tensor_tensor(bp[:], t5[:], t3[:], A.add)

            # matvec coefficients: cq16 = alpha*hd^2 (bf16),
            # cr16 = (beta - 2*alpha*w)*hd (f16)
            ah = f32t("ah", pool=ppool)          # alpha*hd^2
            nc.gpsimd.tensor_tensor(ah[:], t4[:], hd[:], A.mult)
            cq16 = cpool.tile([H, LPC], BF16, tag="cq16")
            nc.scalar.activation(cq16[:], ah[:], ACT.Copy)
            taw = f32t("taw", pool=ppool)        # 2*alpha*w = t5*w'
            nc.gpsimd.tensor_tensor(taw[:], t5[:], wp[:], A.mult)
            bw = f32t("bw", pool=ppool)          # beta - 2*alpha*w
            nc.gpsimd.tensor_tensor(bw[:], bp[:], taw[:], A.subtract)
            bwh = f32t("bwh", pool=ppool)        # (beta-2*alpha*w)*hd
            nc.gpsimd.tensor_tensor(bwh[:], bw[:], hd[:], A.mult)
            # completed square: u = cr/(2 cq); plane rv = rt' + u via
            # max(l1' + wm1 + u, u); contribution cq*rv^2 - cq*u^2 (the
            # constant folds into the ones-matmul base plane A2C2).
            ah2 = f32t("ah2", pool=ppool)        # 2*alpha*hd^2
            nc.vector.tensor_scalar(ah2[:], ah[:], 2.0, None, A.mult)
            rah2 = f32t("rah2", pool=ppool)      # 1/(2 cq)
            nc.vector.reciprocal(rah2[:], ah2[:])
            u_ = f32t("u_")                      # u = cr/(2 cq)
            nc.gpsimd.tensor_tensor(u_[:], bwh[:], rah2[:], A.mult)
            wmv = f32t("wmv")                    # wm1 + u (rt-op s1)
            nc.gpsimd.tensor_tensor(wmv[:], wm1[:], u_[:], A.add)
            uu = f32t("uu", pool=ppool)          # u^2
            nc.gpsimd.tensor_tensor(uu[:], u_[:], u_[:], A.mult)
            cuu = f32t("cuu", pool=ppool)        # cq*u^2
            nc.gpsimd.tensor_tensor(cuu[:], ah[:], uu[:], A.mult)

            # base-term planes (rhs of base matmuls), f32
            A2 = f32t("A2")                      # a^2
            nc.gpsimd.tensor_tensor(A2[:], a_[:], a_[:], A.mult)
            acen = f32t("acen", pool=ppool)
            nc.gpsimd.tensor_tensor(acen[:], a_[:], cen[:], A.mult)
            A2C2 = f32t("A2C2")                  # (a*cen)^2
            nc.gpsimd.tensor_tensor(A2C2[:], acen[:], acen[:], A.mult)
            A2C2v = f32t("A2C2v")                # (a*cen)^2 - cq*u^2
            nc.gpsimd.tensor_tensor(A2C2v[:], A2C2[:], cuu[:], A.subtract)
            t6 = f32t("t6", pool=ppool)
            nc.gpsimd.tensor_tensor(t6[:], A2[:], cen[:], A.mult)
            M2AC = f32t("M2AC")                  # -2*a^2*cen
            nc.vector.tensor_scalar(M2AC[:], t6[:], -2.0, None, A.mult)

            ppool_cm.__exit__(None, None, None)
            lpool = ctx.enter_context(tc.tile_pool(name="l1", bufs=3))
            rpool = ctx.enter_context(tc.tile_pool(name="rg", bufs=2))
            qpool = ctx.enter_context(tc.tile_pool(name="qg", bufs=2))
            # bufs=8: all 8 output chunks pipeline through sqrt/negate/DMA
            # concurrently (bufs=2 serialized the epilogue at ~2.4us/chunk).
            opool = ctx.enter_context(tc.tile_pool(name="outs", bufs=8))
            x2T = cpool.tile([H, B], F32, tag="x2T")   # x^2
            nc.vector.tensor_tensor(x2T[:], xT[:], xT[:], A.mult)
            ones = cpool.tile([H, 128], F32, tag="ones")
            nc.gpsimd.memset(ones[:], 1.0)
            x16 = cpool.tile([H, B], F16, tag="x16")
            nc.vector.tensor_copy(x16[:], xT[:])

            tiles = dict(xT=xT, x2T=x2T, ones=ones, invhd=invhd, cod=cod,
                         ncod=ncod, wmv=wmv, u_=u_, x16=x16, A2=A2,
                         M2AC=M2AC, A2C2=A2C2v, cq16=cq16)
            if repeat > 1:
                with tc.For_i(0, repeat, 1):
                    _run_body(nc, tc, lpool, rpool, qpool, pspool, opool,
                              tiles, out_d, rt_dve, sq_act, abs_act)
            else:
                _run_body(nc, tc, lpool, rpool, qpool, pspool, opool,
                          tiles, out_d, rt_dve, sq_act, abs_act)

    nc.compile()
    return nc


def _run_body(nc, tc, lpool, rpool, qpool, pspool, opool, tiles, out_d,
              rt_dve, sq_act, abs_act):
    xT, x2T, ones = tiles["xT"], tiles["x2T"], tiles["ones"]
    invhd, cod, ncod = tiles["invhd"], tiles["cod"], tiles["ncod"]
    wmv, u_, x16 = tiles["wmv"], tiles["u_"], tiles["x16"]
    A2, M2AC, A2C2 = tiles["A2"], tiles["M2AC"], tiles["A2C2"]
    cq16 = tiles["cq16"]

    # ---- base matmuls into PSUM ----
    psts = []
    for cch in range(NBCH):
        pst = pspool.tile([128, LPC], F32, tag=f"ps{cch}")
        psts.append(pst)
        sl = bass.ts(cch, 128)
        nc.tensor.matmul(pst[:], x2T[:, sl], A2[:],
                         start=True, stop=False, skip_group_check=True)
        nc.tensor.matmul(pst[:], xT[:, sl], M2AC[:],
                         start=False, stop=False, skip_group_check=True)
        nc.tensor.matmul(pst[:], ones[:], A2C2[:],
                         start=False, stop=False, skip_group_check=True)

    # ---- per-label planes + PE reductions ----
    for g in range(LPC // GRP):
        l0 = g * GRP
        l1g = lpool.tile([H, GRP * B], F16, tag="l1g")
        # First GRP-abs_act labels: t' = x/hd - cen/hd (signed, DVE
        # 4x-mode) + one grouped sign-bit AND for |t'|. Trailing abs_act
        # labels get |t'| straight from Act Abs (scale/bias form).
        n_dve = GRP - abs_act
        for j in range(GRP):
            l = l0 + j
            lsl = slice(l, l + 1)
            gsl = slice(j * B, (j + 1) * B)
            if j < n_dve:
                nc.vector.tensor_scalar(l1g[:, gsl], x16[:], invhd[:, lsl],
                                        cod[:, lsl], A.mult, A.subtract)
            else:
                nc.scalar.activation(l1g[:, gsl], xT[:], ACT.Abs,
                                     bias=ncod[:, lsl], scale=invhd[:, lsl])
        if n_dve > 0:
            nc.vector.tensor_scalar(l1g.bitcast(U16)[:, :n_dve * B],
                                    l1g.bitcast(U16)[:, :n_dve * B],
                                    0x7FFF, None, A.bitwise_and)
        # rt' = relu(l1' - 1 + w') per label: first rt_dve labels on DVE,
        # rest on Pool.
        rg = rpool.tile([H, GRP * B], F16, tag="rg")
        for j in range(GRP):
            l = l0 + j
            lsl = slice(l, l + 1)
            gsl = slice(j * B, (j + 1) * B)
            nc.vector.tensor_scalar(rg[:, gsl], l1g[:, gsl],
                                    wmv[:, lsl], u_[:, lsl], A.add, A.max)
        # q = rt'^2: first sq_act labels via grouped Act Square, rest via
        # grouped Pool tensor_tensor mult.
        qg = qpool.tile([H, GRP * B], BF16, tag="qg")
        if sq_act > 0:
            nc.scalar.activation(qg[:, :sq_act * B], rg[:, :sq_act * B],
                                 ACT.Square)
        if sq_act < GRP:
            nc.gpsimd.tensor_tensor(qg[:, sq_act * B:], rg[:, sq_act * B:],
                                    rg[:, sq_act * B:], A.mult)

        for j in range(GRP):
            l = l0 + j
            lsl = slice(l, l + 1)
            last = l == LPC - 1
            for cch in range(NBCH):
                sl = slice(j * B + cch * 128, j * B + (cch + 1) * 128)
                nc.tensor.matmul(psts[cch][:, lsl], qg[:, sl], cq16[:, lsl],
                                 start=False, stop=last,
                                 skip_group_check=True)
        if (g + 1) * GRP == LPC // 2:
            # Left half of every psum chunk is final: drain it now so
            # its sqrt/negate/DMA hide under the remaining groups.
            _epilogue(nc, opool, psts, out_d, 0, LPC // 2)

    # ---- finalize: out = -sqrt(psum); sqrt on Act, negate on DVE.
    # Wave 0 (labels [0, LPC//2)) was emitted mid-loop; finish the rest.
    _epilogue(nc, opool, psts, out_d, LPC // 2, LPC)


_NC_CACHE = None


def _get_nc():
    global _NC_CACHE
    if _NC_CACHE is None:
        _NC_CACHE = build_nc()
    return _NC_CACHE


def kernel(y: np.ndarray, x: np.ndarray) -> np.ndarray:
    y = np.asarray(y, dtype=np.float32)
    x = np.asarray(x, dtype=np.float32)
    assert y.shape == (L, 2 * H) and x.shape == (B, H)

    nc = _get_nc()
    xT = np.ascontiguousarray(x.T)                       # (H, B)
    in_maps = []
    for c in range(N_CORES):
        ys = y[c * LPC:(c + 1) * LPC]
        in_maps.append({
            "xT": xT,
            "mnT": np.ascontiguousarray(ys[:, :H].T),    # (H, LPC)
            "rawT": np.ascontiguousarray(ys[:, H:].T),   # (H, LPC)
        })
    res = bass_utils.run_bass_kernel_spmd(nc, in_maps,
                                          core_ids=list(range(N_CORES)))
    out = np.concatenate([res.results[c]["out"] for c in range(N_CORES)],
                         axis=1)
    return np.ascontiguousarray(out.astype(np.float32))


def _epilogue(nc, opool, psts, out_d, c0, c1):
    n = c1 - c0
    for cch in range(NBCH):
        sq = opool.tile([128, n], F32, tag=f"sq{c0}")
        nc.scalar.activation(sq[:], psts[cch][:, c0:c1], ACT.Sqrt)
        o = opool.tile([128, n], F32, tag=f"o{c0}")
        nc.vector.tensor_scalar(o[:], sq[:], -1.0, None, A.mult)
        nc.sync.dma_start(out_d.ap()[bass.ts(cch, 128), c0:c1], o[:])


# revision 18
# speedup vs baseline: 1.5067x; 1.5067x over previous
"""BoxE scorer kernel v3 for Trainium2 (8 NeuronCores, label-sharded).

Same widened-relu math as v2 (see below), but the per-group elementwise
work is load-balanced across THREE engines instead of two:

  DVE:   8x t'-op (x16*invhd - cod, fp16 4x-mode), grouped sign-bit AND,
         RT_DVE rt-ops                                   (~6.1 us/group)
  Pool:  RT_POOL rt-ops, grouped tensor-tensor square
         over the last (GRP-SQ_ACT) labels               (~5.5 us/group)
  Act:   grouped Square over the first SQ_ACT labels     (~5.4 us/group)
  PE:    2 matvecs per (label, chunk) + base matmuls     (cheap)

v2 ran the whole t'/AND/rt chain on DVE (7.9 us/group) with the full
grouped Square on Act (7.0 us/group); Pool sat idle.

Math ("widened-relu" formulation): with l1 = |x - cen|, hd = d/2,
m = relu(l1 - hd), s = [l1 > hd], the outside correction is
  corr_h = alpha*m^2 + beta*m + gamma*s .
Choosing w > 0 with  -alpha*w^2 + beta*w = gamma  and
rt = relu(l1 - hd + w):
  alpha*rt^2 + (beta - 2*alpha*w)*rt  ==  corr_h
exactly for points outside or deep inside; the band l1 in (hd-w, hd)
picks up a small spurious term bounded by gamma (measured end-to-end
fro err ~4e-3 against the fp64 reference; budget 2e-2).
Per label only
  rt' = relu(l1' - 1 + w')   (dual-op tensor_scalar, vec scalar bias)
  q   = rt'^2                (grouped Square)
plus TWO PE matvecs per (label, chunk) and the base quadratic
sum_h a^2 (x-cen)^2 as 3 dense f32 matmuls.
"""

from contextlib import ExitStack

import numpy as np

import concourse.bass as bass
import concourse.tile as tile
from concourse import bacc, mybir
from concourse import bass_utils

F32 = mybir.dt.float32
F16 = mybir.dt.float16
BF16 = mybir.dt.bfloat16
U16 = mybir.dt.uint16
A = mybir.AluOpType
ACT = mybir.ActivationFunctionType

B = 1024      # batch
H = 128       # hidden
L = 2048      # num labels
N_CORES = 8
LPC = L // N_CORES   # labels per core
NBCH = B // 128      # batch chunks of 128
GRP = 8              # labels per grouped instruction

RT_DVE = 8    # rt-op labels on DVE
SQ_ACT = 5    # square labels on Act grouped Square (rest on Pool tt-mult)
ABS_ACT = 2   # trailing labels whose |t'| comes from Act Abs (skip t'+AND)


def build_nc(repeat: int = 1, rt_dve: int = RT_DVE, sq_act: int = SQ_ACT,
             abs_act: int = ABS_ACT):
    nc = bacc.Bacc("TRN2", target_bir_lowering=False, debug=False,
                   num_devices=N_CORES)
    xT_d = nc.dram_tensor("xT", (H, B), F32, kind="ExternalInput")
    mnT_d = nc.dram_tensor("mnT", (H, LPC), F32, kind="ExternalInput")
    rawT_d = nc.dram_tensor("rawT", (H, LPC), F32, kind="ExternalInput")
    out_d = nc.dram_tensor("out", (B, LPC), F32, kind="ExternalOutput")

    with tile.TileContext(nc) as tc:
        with ExitStack() as ctx:
            cpool = ctx.enter_context(tc.tile_pool(name="consts", bufs=1))
            pspool = ctx.enter_context(
                tc.tile_pool(name="psum", bufs=1, space=bass.MemorySpace.PSUM))

            # ---- load inputs ----
            ppool_cm = tc.tile_pool(name="pre", bufs=1)
            ppool = ppool_cm.__enter__()
            rawT = ppool.tile([H, LPC], F32, tag="rawT")
            nc.sync.dma_start(rawT[:], rawT_d.ap())
            xT = cpool.tile([H, B], F32, tag="xT")
            nc.sync.dma_start(xT[:], xT_d.ap())
            mnT = ppool.tile([H, LPC], F32, tag="mnT")
            nc.sync.dma_start(mnT[:], mnT_d.ap())

            def f32t(tag, pool=None):
                return (pool or cpool).tile([H, LPC], F32, tag=tag, name=tag)

            # ---- per-label coefficients (all [H, LPC] f32) ----
            # Critical-path consts (t'-op needs invhd/cod; rt needs wm1)
            # first so the main loop can start while the rest finishes.
            # Independent chains alternate DVE/Pool.
            e = f32t("e", pool=ppool)
            nc.scalar.activation(e[:], rawT[:], ACT.Exp)
            e1 = f32t("e1", pool=ppool)
            nc.vector.tensor_scalar(e1[:], e[:], 1.0, None, A.add)
            delta = f32t("delta", pool=ppool)     # softplus(raw)
            nc.scalar.activation(delta[:], e1[:], ACT.Ln)

            hd = f32t("hd", pool=ppool)          # d/2
            nc.vector.tensor_scalar(hd[:], delta[:], 0.5, None, A.mult)
            cen = f32t("cen", pool=ppool)        # mn + d/2
            nc.gpsimd.tensor_tensor(cen[:], mnT[:], hd[:], A.add)
            invhd = f32t("invhd")                # 1/hd
            nc.vector.reciprocal(invhd[:], hd[:])
            cod = f32t("cod")                    # cen/hd
            nc.vector.tensor_tensor(cod[:], cen[:], invhd[:], A.mult)
            ncod = f32t("ncod")                  # -cen/hd (Act-Abs bias)
            nc.vector.tensor_scalar(ncod[:], cod[:], -1.0, None, A.mult)

            dp1 = f32t("dp1", pool=ppool)        # bb = d+1
            nc.vector.tensor_scalar(dp1[:], delta[:], 1.0, None, A.add)
            dp1e = f32t("dp1e", pool=ppool)
            nc.vector.tensor_scalar(dp1e[:], dp1[:], 1e-10, None, A.add)
            a_ = f32t("a_", pool=ppool)          # a = 1/(bb+1e-10)
            nc.vector.reciprocal(a_[:], dp1e[:])
            de = f32t("de", pool=ppool)
            nc.vector.tensor_scalar(de[:], delta[:], 1e-10, None, A.add)
            rd = f32t("rd", pool=ppool)          # 1/(d+1e-10)
            nc.vector.reciprocal(rd[:], de[:])

            dmr = f32t("dmr", pool=ppool)        # d - 1/d
            nc.gpsimd.tensor_tensor(dmr[:], delta[:], rd[:], A.subtract)
            nhd = f32t("nhd", pool=ppool)        # -d/2
            nc.vector.tensor_scalar(nhd[:], hd[:], -1.0, None, A.mult)
            c_ = f32t("c_", pool=ppool)          # c = -(d/2)(d - 1/d)
            nc.gpsimd.tensor_tensor(c_[:], dmr[:], nhd[:], A.mult)

            Dl = f32t("Dl", pool=ppool)          # D = bb - a
            nc.gpsimd.tensor_tensor(Dl[:], dp1[:], a_[:], A.subtract)
            abb = f32t("abb", pool=ppool)        # Q = bb + a
            nc.gpsimd.tensor_tensor(abb[:], dp1[:], a_[:], A.add)
            al = f32t("al", pool=ppool)          # alpha = D*Q
            nc.gpsimd.tensor_tensor(al[:], Dl[:], abb[:], A.mult)

            # w = hd + c/Q ;  w' = w/hd ; wm1 = w' - 1 (rt-op bias)
            rq = f32t("rq", pool=ppool)          # 1/Q
            nc.vector.reciprocal(rq[:], abb[:])
            cq_ = f32t("cq_", pool=ppool)        # c/Q
            nc.gpsimd.tensor_tensor(cq_[:], c_[:], rq[:], A.mult)
            w_ = f32t("w_", pool=ppool)          # w = hd + c/Q
            nc.gpsimd.tensor_tensor(w_[:], hd[:], cq_[:], A.add)
            wp = f32t("wp", pool=ppool)          # w' = w/hd
            nc.gpsimd.tensor_tensor(wp[:], w_[:], invhd[:], A.mult)
            wm1 = f32t("wm1")                    # w' - 1
            nc.vector.tensor_scalar(wm1[:], wp[:], 1.0, None, A.subtract)

            t2 = f32t("t2", pool=ppool)          # D + Q
            nc.gpsimd.tensor_tensor(t2[:], Dl[:], abb[:], A.add)
            t3 = f32t("t3", pool=ppool)          # c*(D+Q)
            nc.gpsimd.tensor_tensor(t3[:], t2[:], c_[:], A.mult)
            t4 = f32t("t4", pool=ppool)          # alpha*hd
            nc.gpsimd.tensor_tensor(t4[:], al[:], hd[:], A.mult)
            t5 = f32t("t5", pool=ppool)          # 2*alpha*hd
            nc.vector.tensor_scalar(t5[:], t4[:], 2.0, None, A.mult)
            bp = f32t("bp", pool=ppool)          # beta
            nc.gpsimd.tensor_tensor(bp[:], t5[:], t3[:], A.add)

            # matvec coefficients: cq16 = alpha*hd^2 (bf16),
            # cr16 = (beta - 2*alpha*w)*hd (f16)
            ah = f32t("ah", pool=ppool)          # alpha*hd^2
            nc.gpsimd.tensor_tensor(ah[:], t4[:], hd[:], A.mult)
            cq16 = cpool.tile([H, LPC], BF16, tag="cq16")
            nc.scalar.activation(cq16[:], ah[:], ACT.Copy)
            taw = f32t("taw", pool=ppool)        # 2*alpha*w = t5*w'
            nc.gpsimd.tensor_tensor(taw[:], t5[:], wp[:], A.mult)
            bw = f32t("bw", pool=ppool)          # beta - 2*alpha*w
            nc.gpsimd.tensor_tensor(bw[:], bp[:], taw[:], A.subtract)
            bwh = f32t("bwh", pool=ppool)        # (beta-2*alpha*w)*hd
            nc.gpsimd.tensor_tensor(bwh[:], bw[:], hd[:], A.mult)
            # completed square: u = cr/(2 cq); plane rv = rt' + u via
            # max(l1' + wm1 + u, u); contribution cq*rv^2 - cq*u^2 (the
            # constant folds into the ones-matmul base plane A2C2).
            ah2 = f32t("ah2", pool=ppool)        # 2*alpha*hd^2
            nc.vector.tensor_scalar(ah2[:], ah[:], 2.0, None, A.mult)
            rah2 = f32t("rah2", pool=ppool)      # 1/(2 cq)
            nc.vector.reciprocal(rah2[:], ah2[:])
            u_ = f32t("u_")                      # u = cr/(2 cq)
            nc.gpsimd.tensor_tensor(u_[:], bwh[:], rah2[:], A.mult)
            wmv = f32t("wmv")                    # wm1 + u (rt-op s1)
            nc.gpsimd.tensor_tensor(wmv[:], wm1[:], u_[:], A.add)
            uu = f32t("uu", pool=ppool)          # u^2
            nc.gpsimd.tensor_tensor(uu[:], u_[:], u_[:], A.mult)
            cuu = f32t("cuu", pool=ppool)        # cq*u^2
            nc.gpsimd.tensor_tensor(cuu[:], ah[:], uu[:], A.mult)

            # base-term planes (rhs of base matmuls), f32
            A2 = f32t("A2")                      # a^2
            nc.gpsimd.tensor_tensor(A2[:], a_[:], a_[:], A.mult)
            acen = f32t("acen", pool=ppool)
            nc.gpsimd.tensor_tensor(acen[:], a_[:], cen[:], A.mult)
            A2C2 = f32t("A2C2")                  # (a*cen)^2
            nc.gpsimd.tensor_tensor(A2C2[:], acen[:], acen[:], A.mult)
            A2C2v = f32t("A2C2v")                # (a*cen)^2 - cq*u^2
            nc.gpsimd.tensor_tensor(A2C2v[:], A2C2[:], cuu[:], A.subtract)
            t6 = f32t("t6", pool=ppool)
            nc.gpsimd.tensor_tensor(t6[:], A2[:], cen[:], A.mult)
            M2AC = f32t("M2AC")                  # -2*a^2*cen
            nc.vector.tensor_scalar(M2AC[:], t6[:], -2.0, None, A.mult)

            ppool_cm.__exit__(None, None, None)
            lpool = ctx.enter_context(tc.tile_pool(name="l1", bufs=3))
            rpool = ctx.enter_context(tc.tile_pool(name="rg", bufs=2))
            qpool = ctx.enter_context(tc.tile_pool(name="qg", bufs=2))
            # bufs=8: all 8 output chunks pipeline through sqrt/negate/DMA
            # concurrently (bufs=2 serialized the epilogue at ~2.4us/chunk).
            opool = ctx.enter_context(tc.tile_pool(name="outs", bufs=8))
            x2T = cpool.tile([H, B], F32, tag="x2T")   # x^2
            nc.vector.tensor_tensor(x2T[:], xT[:], xT[:], A.mult)
            ones = cpool.tile([H, 128], F32, tag="ones")
            nc.gpsimd.memset(ones[:], 1.0)
            x16 = cpool.tile([H, B], F16, tag="x16")
            nc.vector.tensor_copy(x16[:], xT[:])

            tiles = dict(xT=xT, x2T=x2T, ones=ones, invhd=invhd, cod=cod,
                         ncod=ncod, wmv=wmv, u_=u_, x16=x16, A2=A2,
                         M2AC=M2AC, A2C2=A2C2v, cq16=cq16)
            if repeat > 1:
                with tc.For_i(0, repeat, 1):
                    _run_body(nc, tc, lpool, rpool, qpool, pspool, opool,
                              tiles, out_d, rt_dve, sq_act, abs_act)
            else:
                _run_body(nc, tc, lpool, rpool, qpool, pspool, opool,
                          tiles, out_d, rt_dve, sq_act, abs_act)

    nc.compile()
    return nc


def _run_body(nc, tc, lpool, rpool, qpool, pspool, opool, tiles, out_d,
              rt_dve, sq_act, abs_act):
    xT, x2T, ones = tiles["xT"], tiles["x2T"], tiles["ones"]
    invhd, cod, ncod = tiles["invhd"], tiles["cod"], tiles["ncod"]
    wmv, u_, x16 = tiles["wmv"], tiles["u_"], tiles["x16"]
    A2, M2AC, A2C2 = tiles["A2"], tiles["M2AC"], tiles["A2C2"]
    cq16 = tiles["cq16"]

    # ---- base matmuls into PSUM ----
    psts = []
    for cch in range(NBCH):
        pst = pspool.tile([128, LPC], F32, tag=f"ps{cch}")
        psts.append(pst)
        sl = bass.ts(cch, 128)
        nc.tensor.matmul(pst[:], x2T[:, sl], A2[:],
                         start=True, stop=False, skip_group_check=True)
        nc.tensor.matmul(pst[:], xT[:, sl], M2AC[:],
                         start=False, stop=False, skip_group_check=True)
        nc.tensor.matmul(pst[:], ones[:], A2C2[:],
                         start=False, stop=False, skip_group_check=True)

    # ---- per-label planes + PE reductions ----
    for g in range(LPC // GRP):
        l0 = g * GRP
        l1g = lpool.tile([H, GRP * B], F16, tag="l1g")
        # First GRP-abs_act labels: t' = x/hd - cen/hd (signed, DVE
        # 4x-mode) + one grouped sign-bit AND for |t'|. Trailing abs_act
        # labels get |t'| straight from Act Abs (scale/bias form).
        n_dve = GRP - abs_act
        for j in range(GRP):
            l = l0 + j
            lsl = slice(l, l + 1)
            gsl = slice(j * B, (j + 1) * B)
            if j < n_dve:
                nc.vector.tensor_scalar(l1g[:, gsl], x16[:], invhd[:, lsl],
                                        cod[:, lsl], A.mult, A.subtract)
            else:
                nc.scalar.activation(l1g[:, gsl], xT[:], ACT.Abs,
                                     bias=ncod[:, lsl], scale=invhd[:, lsl])
        if n_dve > 0:
            nc.vector.tensor_scalar(l1g.bitcast(U16)[:, :n_dve * B],
                                    l1g.bitcast(U16)[:, :n_dve * B],
                                    0x7FFF, None, A.bitwise_and)
        # rt' = relu(l1' - 1 + w') per label: first rt_dve labels on DVE,
        # rest on Pool.
        rg = rpool.tile([H, GRP * B], F16, tag="rg")
        for j in range(GRP):
            l = l0 + j
            lsl = slice(l, l + 1)
            gsl = slice(j * B, (j + 1) * B)
            nc.vector.tensor_scalar(rg[:, gsl], l1g[:, gsl],
                                    wmv[:, lsl], u_[:, lsl], A.add, A.max)
        # q = rt'^2: first sq_act labels via grouped Act Square, rest via
        # grouped Pool tensor_tensor mult.
        qg = qpool.tile([H, GRP * B], BF16, tag="qg")
        if sq_act > 0:
            nc.scalar.activation(qg[:, :sq_act * B], rg[:, :sq_act * B],
                                 ACT.Square)
        if sq_act < GRP:
            # label sq_act on DVE; the rest as separate 1-label Pool
            # tensor_tensor ops (HW Pool is ~3x the model's rate, and
            # worse for wide ops — keep each at 1024 free elems).
            nc.vector.tensor_tensor(qg[:, sq_act * B:(sq_act + 1) * B],
                                    rg[:, sq_act * B:(sq_act + 1) * B],
                                    rg[:, sq_act * B:(sq_act + 1) * B],
                                    A.mult)
            for j in range(sq_act + 1, GRP):
                nc.gpsimd.tensor_tensor(qg[:, j * B:(j + 1) * B],
                                        rg[:, j * B:(j + 1) * B],
                                        rg[:, j * B:(j + 1) * B], A.mult)

        for j in range(GRP):
            l = l0 + j
            lsl = slice(l, l + 1)
            last = l == LPC - 1
            for cch in range(NBCH):
                sl = slice(j * B + cch * 128, j * B + (cch + 1) * 128)
                nc.tensor.matmul(psts[cch][:, lsl], qg[:, sl], cq16[:, lsl],
                                 start=False, stop=last,
                                 skip_group_check=True)

    # ---- finalize: out = -sqrt(psum); sqrt on Act, negate on DVE ----
    _epilogue(nc, opool, psts, out_d, 0, LPC)


_NC_CACHE = None


def _get_nc():
    global _NC_CACHE
    if _NC_CACHE is None:
        _NC_CACHE = build_nc()
    return _NC_CACHE


def kernel(y: np.ndarray, x: np.ndarray) -> np.ndarray:
    y = np.asarray(y, dtype=np.float32)
    x = np.asarray(x, dtype=np.float32)
    assert y.shape == (L, 2 * H) and x.shape == (B, H)

    nc = _get_nc()
    xT = np.ascontiguousarray(x.T)                       # (H, B)
    in_maps = []
    for c in range(N_CORES):
        ys = y[c * LPC:(c + 1) * LPC]
        in_maps.append({
            "xT": xT,
            "mnT": np.ascontiguousarray(ys[:, :H].T),    # (H, LPC)
            "rawT": np.ascontiguousarray(ys[:, H:].T),   # (H, LPC)
        })
    res = bass_utils.run_bass_kernel_spmd(nc, in_maps,
                                          core_ids=list(range(N_CORES)))
    out = np.concatenate([res.results[c]["out"] for c in range(N_CORES)],
                         axis=1)
    return np.ascontiguousarray(out.astype(np.float32))


def _epilogue(nc, opool, psts, out_d, c0, c1):
    n = c1 - c0
    for cch in range(NBCH):
        sq = opool.tile([128, n], F32, tag=f"sq{c0}")
        nc.scalar.activation(sq[:], psts[cch][:, c0:c1], ACT.Sqrt)
        o = opool.tile([128, n], F32, tag=f"o{c0}")
        nc.vector.tensor_scalar(o[:], sq[:], -1.0, None, A.mult)
        nc.sync.dma_start(out_d.ap()[bass.ts(cch, 128), c0:c1], o[:])


# revision 19
# speedup vs baseline: 1.8667x; 1.2389x over previous
"""BoxE scorer kernel v3 for Trainium2 (8 NeuronCores, label-sharded).

Same widened-relu math as v2 (see below), but the per-group elementwise
work is load-balanced across THREE engines instead of two:

  DVE:   8x t'-op (x16*invhd - cod, fp16 4x-mode), grouped sign-bit AND,
         RT_DVE rt-ops                                   (~6.1 us/group)
  Pool:  RT_POOL rt-ops, grouped tensor-tensor square
         over the last (GRP-SQ_ACT) labels               (~5.5 us/group)
  Act:   grouped Square over the first SQ_ACT labels     (~5.4 us/group)
  PE:    2 matvecs per (label, chunk) + base matmuls     (cheap)

v2 ran the whole t'/AND/rt chain on DVE (7.9 us/group) with the full
grouped Square on Act (7.0 us/group); Pool sat idle.

Math ("widened-relu" formulation): with l1 = |x - cen|, hd = d/2,
m = relu(l1 - hd), s = [l1 > hd], the outside correction is
  corr_h = alpha*m^2 + beta*m + gamma*s .
Choosing w > 0 with  -alpha*w^2 + beta*w = gamma  and
rt = relu(l1 - hd + w):
  alpha*rt^2 + (beta - 2*alpha*w)*rt  ==  corr_h
exactly for points outside or deep inside; the band l1 in (hd-w, hd)
picks up a small spurious term bounded by gamma (measured end-to-end
fro err ~4e-3 against the fp64 reference; budget 2e-2).
Per label only
  rt' = relu(l1' - 1 + w')   (dual-op tensor_scalar, vec scalar bias)
  q   = rt'^2                (grouped Square)
plus TWO PE matvecs per (label, chunk) and the base quadratic
sum_h a^2 (x-cen)^2 as 3 dense f32 matmuls.
"""

from contextlib import ExitStack

import numpy as np

import concourse.bass as bass
import concourse.tile as tile
from concourse import bacc, mybir
from concourse import bass_utils

F32 = mybir.dt.float32
F16 = mybir.dt.float16
BF16 = mybir.dt.bfloat16
U16 = mybir.dt.uint16
A = mybir.AluOpType
ACT = mybir.ActivationFunctionType

B = 1024      # batch
H = 128       # hidden
L = 2048      # num labels
N_CORES = 8
LPC = L // N_CORES   # labels per core
NBCH = B // 128      # batch chunks of 128
GRP = 8              # labels per grouped instruction

RT_DVE = 8    # rt-op labels on DVE
SQ_ACT = 7    # square labels on Act grouped Square (rest on Pool tt-mult)
ABS_ACT = 1   # trailing labels whose |t'| comes from Act Abs (skip t'+AND)


def build_nc(repeat: int = 1, rt_dve: int = RT_DVE, sq_act: int = SQ_ACT,
             abs_act: int = ABS_ACT):
    nc = bacc.Bacc("TRN2", target_bir_lowering=False, debug=False,
                   num_devices=N_CORES)
    xT_d = nc.dram_tensor("xT", (H, B), F32, kind="ExternalInput")
    mnT_d = nc.dram_tensor("mnT", (H, LPC), F32, kind="ExternalInput")
    rawT_d = nc.dram_tensor("rawT", (H, LPC), F32, kind="ExternalInput")
    out_d = nc.dram_tensor("out", (B, LPC), F32, kind="ExternalOutput")

    with tile.TileContext(nc) as tc:
        with ExitStack() as ctx:
            cpool = ctx.enter_context(tc.tile_pool(name="consts", bufs=1))
            pspool = ctx.enter_context(
                tc.tile_pool(name="psum", bufs=1, space=bass.MemorySpace.PSUM))

            # ---- load inputs ----
            ppool_cm = tc.tile_pool(name="pre", bufs=1)
            ppool = ppool_cm.__enter__()
            rawT = ppool.tile([H, LPC], F32, tag="rawT")
            nc.sync.dma_start(rawT[:], rawT_d.ap())
            xT = cpool.tile([H, B], F32, tag="xT")
            nc.sync.dma_start(xT[:], xT_d.ap())
            mnT = ppool.tile([H, LPC], F32, tag="mnT")
            nc.sync.dma_start(mnT[:], mnT_d.ap())

            def f32t(tag, pool=None):
                return (pool or cpool).tile([H, LPC], F32, tag=tag, name=tag)

            # ---- per-label coefficients (all [H, LPC] f32) ----
            # Critical-path consts (t'-op needs invhd/cod; rt needs wm1)
            # first so the main loop can start while the rest finishes.
            # Independent chains alternate DVE/Pool.
            e = f32t("e", pool=ppool)
            nc.scalar.activation(e[:], rawT[:], ACT.Exp)
            e1 = f32t("e1", pool=ppool)
            nc.vector.tensor_scalar(e1[:], e[:], 1.0, None, A.add)
            delta = f32t("delta", pool=ppool)     # softplus(raw)
            nc.scalar.activation(delta[:], e1[:], ACT.Ln)

            hd = f32t("hd", pool=ppool)          # d/2
            nc.vector.tensor_scalar(hd[:], delta[:], 0.5, None, A.mult)
            cen = f32t("cen", pool=ppool)        # mn + d/2
            nc.vector.tensor_tensor(cen[:], mnT[:], hd[:], A.add)
            invhd = f32t("invhd")                # 1/hd
            nc.vector.reciprocal(invhd[:], hd[:])
            cod = f32t("cod")                    # cen/hd
            nc.vector.tensor_tensor(cod[:], cen[:], invhd[:], A.mult)
            ncod = f32t("ncod")                  # -cen/hd (Act-Abs bias)
            nc.vector.tensor_scalar(ncod[:], cod[:], -1.0, None, A.mult)

            dp1 = f32t("dp1", pool=ppool)        # bb = d+1
            nc.vector.tensor_scalar(dp1[:], delta[:], 1.0, None, A.add)
            dp1e = f32t("dp1e", pool=ppool)
            nc.vector.tensor_scalar(dp1e[:], dp1[:], 1e-10, None, A.add)
            a_ = f32t("a_", pool=ppool)          # a = 1/(bb+1e-10)
            nc.vector.reciprocal(a_[:], dp1e[:])
            de = f32t("de", pool=ppool)
            nc.vector.tensor_scalar(de[:], delta[:], 1e-10, None, A.add)
            rd = f32t("rd", pool=ppool)          # 1/(d+1e-10)
            nc.vector.reciprocal(rd[:], de[:])

            dmr = f32t("dmr", pool=ppool)        # d - 1/d
            nc.vector.tensor_tensor(dmr[:], delta[:], rd[:], A.subtract)
            nhd = f32t("nhd", pool=ppool)        # -d/2
            nc.vector.tensor_scalar(nhd[:], hd[:], -1.0, None, A.mult)
            c_ = f32t("c_", pool=ppool)          # c = -(d/2)(d - 1/d)
            nc.vector.tensor_tensor(c_[:], dmr[:], nhd[:], A.mult)

            Dl = f32t("Dl", pool=ppool)          # D = bb - a
            nc.vector.tensor_tensor(Dl[:], dp1[:], a_[:], A.subtract)
            abb = f32t("abb", pool=ppool)        # Q = bb + a
            nc.vector.tensor_tensor(abb[:], dp1[:], a_[:], A.add)
            al = f32t("al", pool=ppool)          # alpha = D*Q
            nc.vector.tensor_tensor(al[:], Dl[:], abb[:], A.mult)

            # w = hd + c/Q ;  w' = w/hd ; wm1 = w' - 1 (rt-op bias)
            rq = f32t("rq", pool=ppool)          # 1/Q
            nc.vector.reciprocal(rq[:], abb[:])
            cq_ = f32t("cq_", pool=ppool)        # c/Q
            nc.vector.tensor_tensor(cq_[:], c_[:], rq[:], A.mult)
            w_ = f32t("w_", pool=ppool)          # w = hd + c/Q
            nc.vector.tensor_tensor(w_[:], hd[:], cq_[:], A.add)
            wp = f32t("wp", pool=ppool)          # w' = w/hd
            nc.vector.tensor_tensor(wp[:], w_[:], invhd[:], A.mult)
            wm1 = f32t("wm1")                    # w' - 1
            nc.vector.tensor_scalar(wm1[:], wp[:], 1.0, None, A.subtract)

            t2 = f32t("t2", pool=ppool)          # D + Q
            nc.vector.tensor_tensor(t2[:], Dl[:], abb[:], A.add)
            t3 = f32t("t3", pool=ppool)          # c*(D+Q)
            nc.vector.tensor_tensor(t3[:], t2[:], c_[:], A.mult)
            t4 = f32t("t4", pool=ppool)          # alpha*hd
            nc.vector.tensor_tensor(t4[:], al[:], hd[:], A.mult)
            t5 = f32t("t5", pool=ppool)          # 2*alpha*hd
            nc.vector.tensor_scalar(t5[:], t4[:], 2.0, None, A.mult)
            bp = f32t("bp", pool=ppool)          # beta
            nc.vector.tensor_tensor(bp[:], t5[:], t3[:], A.add)

            # matvec coefficients: cq16 = alpha*hd^2 (bf16),
            # cr16 = (beta - 2*alpha*w)*hd (f16)
            ah = f32t("ah", pool=ppool)          # alpha*hd^2
            nc.vector.tensor_tensor(ah[:], t4[:], hd[:], A.mult)
            cq16 = cpool.tile([H, LPC], BF16, tag="cq16")
            nc.scalar.activation(cq16[:], ah[:], ACT.Copy)
            taw = f32t("taw", pool=ppool)        # 2*alpha*w = t5*w'
            nc.vector.tensor_tensor(taw[:], t5[:], wp[:], A.mult)
            bw = f32t("bw", pool=ppool)          # beta - 2*alpha*w
            nc.vector.tensor_tensor(bw[:], bp[:], taw[:], A.subtract)
            bwh = f32t("bwh", pool=ppool)        # (beta-2*alpha*w)*hd
            nc.vector.tensor_tensor(bwh[:], bw[:], hd[:], A.mult)
            # completed square: u = cr/(2 cq); plane rv = rt' + u via
            # max(l1' + wm1 + u, u); contribution cq*rv^2 - cq*u^2 (the
            # constant folds into the ones-matmul base plane A2C2).
            ah2 = f32t("ah2", pool=ppool)        # 2*alpha*hd^2
            nc.vector.tensor_scalar(ah2[:], ah[:], 2.0, None, A.mult)
            rah2 = f32t("rah2", pool=ppool)      # 1/(2 cq)
            nc.vector.reciprocal(rah2[:], ah2[:])
            u_ = f32t("u_")                      # u = cr/(2 cq)
            nc.vector.tensor_tensor(u_[:], bwh[:], rah2[:], A.mult)
            wmv = f32t("wmv")                    # wm1 + u (rt-op s1)
            nc.vector.tensor_tensor(wmv[:], wm1[:], u_[:], A.add)
            uu = f32t("uu", pool=ppool)          # u^2
            nc.vector.tensor_tensor(uu[:], u_[:], u_[:], A.mult)
            cuu = f32t("cuu", pool=ppool)        # cq*u^2
            nc.vector.tensor_tensor(cuu[:], ah[:], uu[:], A.mult)

            # base-term planes (rhs of base matmuls), f32
            A2 = f32t("A2")                      # a^2
            nc.vector.tensor_tensor(A2[:], a_[:], a_[:], A.mult)
            acen = f32t("acen", pool=ppool)
            nc.vector.tensor_tensor(acen[:], a_[:], cen[:], A.mult)
            A2C2 = f32t("A2C2")                  # (a*cen)^2
            nc.vector.tensor_tensor(A2C2[:], acen[:], acen[:], A.mult)
            A2C2v = f32t("A2C2v")                # (a*cen)^2 - cq*u^2
            nc.vector.tensor_tensor(A2C2v[:], A2C2[:], cuu[:], A.subtract)
            t6 = f32t("t6", pool=ppool)
            nc.vector.tensor_tensor(t6[:], A2[:], cen[:], A.mult)
            M2AC = f32t("M2AC")                  # -2*a^2*cen
            nc.vector.tensor_scalar(M2AC[:], t6[:], -2.0, None, A.mult)

            ppool_cm.__exit__(None, None, None)
            lpool = ctx.enter_context(tc.tile_pool(name="l1", bufs=3))
            rpool = ctx.enter_context(tc.tile_pool(name="rg", bufs=2))
            qpool = ctx.enter_context(tc.tile_pool(name="qg", bufs=2))
            # bufs=8: all 8 output chunks pipeline through sqrt/negate/DMA
            # concurrently (bufs=2 serialized the epilogue at ~2.4us/chunk).
            opool = ctx.enter_context(tc.tile_pool(name="outs", bufs=8))
            x2T = cpool.tile([H, B], F32, tag="x2T")   # x^2
            nc.vector.tensor_tensor(x2T[:], xT[:], xT[:], A.mult)
            ones = cpool.tile([H, 128], F32, tag="ones")
            nc.gpsimd.memset(ones[:], 1.0)
            x16 = cpool.tile([H, B], F16, tag="x16")
            nc.vector.tensor_copy(x16[:], xT[:])

            tiles = dict(xT=xT, x2T=x2T, ones=ones, invhd=invhd, cod=cod,
                         ncod=ncod, wmv=wmv, u_=u_, x16=x16, A2=A2,
                         M2AC=M2AC, A2C2=A2C2v, cq16=cq16)
            if repeat > 1:
                with tc.For_i(0, repeat, 1):
                    _run_body(nc, tc, lpool, rpool, qpool, pspool, opool,
                              tiles, out_d, rt_dve, sq_act, abs_act)
            else:
                _run_body(nc, tc, lpool, rpool, qpool, pspool, opool,
                          tiles, out_d, rt_dve, sq_act, abs_act)

    nc.compile()
    return nc


def _run_body(nc, tc, lpool, rpool, qpool, pspool, opool, tiles, out_d,
              rt_dve, sq_act, abs_act):
    xT, x2T, ones = tiles["xT"], tiles["x2T"], tiles["ones"]
    invhd, cod, ncod = tiles["invhd"], tiles["cod"], tiles["ncod"]
    wmv, u_, x16 = tiles["wmv"], tiles["u_"], tiles["x16"]
    A2, M2AC, A2C2 = tiles["A2"], tiles["M2AC"], tiles["A2C2"]
    cq16 = tiles["cq16"]

    # ---- base matmuls into PSUM ----
    psts = []
    for cch in range(NBCH):
        pst = pspool.tile([128, LPC], F32, tag=f"ps{cch}")
        psts.append(pst)
        sl = bass.ts(cch, 128)
        nc.tensor.matmul(pst[:], x2T[:, sl], A2[:],
                         start=True, stop=False, skip_group_check=True)
        nc.tensor.matmul(pst[:], xT[:, sl], M2AC[:],
                         start=False, stop=False, skip_group_check=True)
        nc.tensor.matmul(pst[:], ones[:], A2C2[:],
                         start=False, stop=False, skip_group_check=True)

    # ---- per-label planes + PE reductions ----
    for g in range(LPC // GRP):
        l0 = g * GRP
        l1g = lpool.tile([H, GRP * B], F16, tag="l1g")
        # First GRP-abs_act labels: t' = x/hd - cen/hd (signed, DVE
        # 4x-mode) + one grouped sign-bit AND for |t'|. Trailing abs_act
        # labels get |t'| straight from Act Abs (scale/bias form).
        n_dve = GRP - abs_act
        for j in range(GRP):
            l = l0 + j
            lsl = slice(l, l + 1)
            gsl = slice(j * B, (j + 1) * B)
            if j < n_dve:
                nc.vector.tensor_scalar(l1g[:, gsl], x16[:], invhd[:, lsl],
                                        cod[:, lsl], A.mult, A.subtract)
            else:
                nc.scalar.activation(l1g[:, gsl], xT[:], ACT.Abs,
                                     bias=ncod[:, lsl], scale=invhd[:, lsl])
        if n_dve > 0:
            nc.vector.tensor_scalar(l1g.bitcast(U16)[:, :n_dve * B],
                                    l1g.bitcast(U16)[:, :n_dve * B],
                                    0x7FFF, None, A.bitwise_and)
        # rt' = relu(l1' - 1 + w') per label: first rt_dve labels on DVE,
        # rest on Pool.
        rg = rpool.tile([H, GRP * B], F16, tag="rg")
        for j in range(GRP):
            l = l0 + j
            lsl = slice(l, l + 1)
            gsl = slice(j * B, (j + 1) * B)
            nc.vector.tensor_scalar(rg[:, gsl], l1g[:, gsl],
                                    wmv[:, lsl], u_[:, lsl], A.add, A.max)
        # q = rt'^2: first sq_act labels via grouped Act Square, rest via
        # grouped Pool tensor_tensor mult.
        qg = qpool.tile([H, GRP * B], BF16, tag="qg")
        if sq_act > 0:
            nc.scalar.activation(qg[:, :sq_act * B], rg[:, :sq_act * B],
                                 ACT.Square)
        if sq_act < GRP:
            # Remaining squares on DVE tensor_tensor. The Pool engine is
            # NOT used anywhere in the steady-state loop: on this HW its
            # tensor ops run ~3-15x slower than the cost model claims
            # (ucode path), so any Pool op becomes the bottleneck.
            nc.vector.tensor_tensor(qg[:, sq_act * B:], rg[:, sq_act * B:],
                                    rg[:, sq_act * B:], A.mult)

        for j in range(GRP):
            l = l0 + j
            lsl = slice(l, l + 1)
            last = l == LPC - 1
            for cch in range(NBCH):
                sl = slice(j * B + cch * 128, j * B + (cch + 1) * 128)
                nc.tensor.matmul(psts[cch][:, lsl], qg[:, sl], cq16[:, lsl],
                                 start=False, stop=last,
                                 skip_group_check=True)

    # ---- finalize: out = -sqrt(psum); sqrt on Act, negate on DVE ----
    _epilogue(nc, opool, psts, out_d, 0, LPC)


_NC_CACHE = None


def _get_nc():
    global _NC_CACHE
    if _NC_CACHE is None:
        _NC_CACHE = build_nc()
    return _NC_CACHE


def kernel(y: np.ndarray, x: np.ndarray) -> np.ndarray:
    y = np.asarray(y, dtype=np.float32)
    x = np.asarray(x, dtype=np.float32)
    assert y.shape == (L, 2 * H) and x.shape == (B, H)

    nc = _get_nc()
    xT = np.ascontiguousarray(x.T)                       # (H, B)
    in_maps = []
    for c in range(N_CORES):
        ys = y[c * LPC:(c + 1) * LPC]
        in_maps.append({
            "xT": xT,
            "mnT": np.ascontiguousarray(ys[:, :H].T),    # (H, LPC)
            "rawT": np.ascontiguousarray(ys[:, H:].T),   # (H, LPC)
        })
    res = bass_utils.run_bass_kernel_spmd(nc, in_maps,
                                          core_ids=list(range(N_CORES)))
    out = np.concatenate([res.results[c]["out"] for c in range(N_CORES)],
                         axis=1)
    return np.ascontiguousarray(out.astype(np.float32))


def _epilogue(nc, opool, psts, out_d, c0, c1):
    n = c1 - c0
    for cch in range(NBCH):
        sq = opool.tile([128, n], F32, tag=f"sq{c0}")
        nc.scalar.activation(sq[:], psts[cch][:, c0:c1], ACT.Sqrt)
        o = opool.tile([128, n], F32, tag=f"o{c0}")
        nc.vector.tensor_scalar(o[:], sq[:], -1.0, None, A.mult)
        nc.sync.dma_start(out_d.ap()[bass.ts(cch, 128), c0:c1], o[:])
